# revision 1
# baseline (speedup 1.0000x reference)
import sys, os
sys.path.insert(0, "/opt/trn_rl_repo")
import numpy as np
import ml_dtypes
_bf16np = ml_dtypes.bfloat16

from contextlib import ExitStack
import concourse.tile as tile
from concourse import bass, bacc, mybir
from concourse.bass import IndirectOffsetOnAxis
from concourse.bass_utils import run_bass_kernel_spmd

N = 50000
P = 128
NCORES = 8
NPC = N // NCORES          # 6250 nodes per core
TPC = (NPC + P - 1) // P   # 49 node tiles per core
NPAD = TPC * P             # 6272 padded nodes per core
NFULL = NCORES * NPAD      # 50176 padded gather-source rows
D = 128
DOUT = 40

_cache = {}


def _build(K, T):
    nc = bacc.Bacc("TRN2", target_bir_lowering=False, debug=False,
                   num_devices=NCORES)
    f32, i32, bf16 = mybir.dt.float32, mybir.dt.int32, mybir.dt.bfloat16

    feat = nc.dram_tensor("feat", [NPAD, D], bf16, kind="ExternalInput").ap()
    srcd = nc.dram_tensor("srcd", [P, T], i32, kind="ExternalInput").ap()
    dstd = nc.dram_tensor("dstd", [P, T], f32, kind="ExternalInput").ap()
    normd = nc.dram_tensor("normd", [P, TPC], f32, kind="ExternalInput").ap()
    iotad = nc.dram_tensor("iotad", [P, P], f32, kind="ExternalInput").ap()
    wd = [nc.dram_tensor(f"w{i}", [D, D if i < 3 else DOUT], f32,
                         kind="ExternalInput").ap() for i in range(4)]
    outd = nc.dram_tensor("out", [NPAD, DOUT], f32, kind="ExternalOutput").ap()

    with tile.TileContext(nc) as tc, ExitStack() as ctx:
        dram = ctx.enter_context(tc.tile_pool(name="dram", bufs=2, space="DRAM"))
        consts = ctx.enter_context(tc.tile_pool(name="consts", bufs=1))
        hpool = ctx.enter_context(tc.tile_pool(name="hs", bufs=2))
        msgp = ctx.enter_context(tc.tile_pool(name="msg", bufs=24))
        selp = ctx.enter_context(tc.tile_pool(name="sel", bufs=24))
        aggp = ctx.enter_context(tc.tile_pool(name="agg", bufs=4))
        outp = ctx.enter_context(tc.tile_pool(name="outp", bufs=4))
        ps1 = ctx.enter_context(tc.tile_pool(name="ps1", bufs=3, space="PSUM"))
        ps2 = ctx.enter_context(tc.tile_pool(name="ps2", bufs=3, space="PSUM"))

        src_sb = consts.tile([P, T], i32)
        nc.gpsimd.dma_start(src_sb[:], srcd[:])
        dst_sb = consts.tile([P, T], f32)
        nc.gpsimd.dma_start(dst_sb[:], dstd[:])
        norm_sb = consts.tile([P, TPC], f32)
        nc.gpsimd.dma_start(norm_sb[:], normd[:])
        iota_sb = consts.tile([P, P], f32)
        nc.gpsimd.dma_start(iota_sb[:], iotad[:])
        w_sb = []
        for i in range(4):
            dcol = D if i < 3 else DOUT
            w = consts.tile([P, dcol], f32)
            nc.gpsimd.dma_start(w[:], wd[i][:])
            w_sb.append(w)

        h_scaled = None
        for layer in range(4):
            dcol = D if layer < 3 else DOUT
            bounce = dram.tile([NPAD, D], bf16)
            hfull = dram.tile([NFULL, D], bf16)
            if layer == 0:
                for t in range(TPC):
                    ft = msgp.tile([P, D], bf16)
                    nc.gpsimd.dma_start(ft[:], feat[t * P:(t + 1) * P, :])
                    nc.gpsimd.dma_start(bounce[t * P:(t + 1) * P, :], ft[:])
            else:
                for t in range(TPC):
                    nc.gpsimd.dma_start(bounce[t * P:(t + 1) * P, :],
                                        h_scaled[:, t * P:(t + 1) * P])
            nc.gpsimd.collective_compute(
                "AllGather", mybir.AluOpType.bypass,
                replica_groups=[list(range(NCORES))],
                ins=[bounce.opt()], outs=[hfull.opt()],
            )
            if layer < 3:
                h_next = hpool.tile([P, NPAD], bf16)
            e = 0
            for t in range(TPC):
                psA = ps1.tile([P, P], f32, space="PSUM")
                kt = K[t]
                for k in range(kt):
                    msg = msgp.tile([P, D], bf16)
                    nc.gpsimd.indirect_dma_start(
                        out=msg[:], out_offset=None, in_=hfull[:],
                        in_offset=IndirectOffsetOnAxis(ap=src_sb[:, e:e + 1],
                                                       axis=0))
                    sel = selp.tile([P, P], bf16)
                    nc.vector.tensor_tensor(
                        out=sel[:],
                        in0=dst_sb[:, e:e + 1].to_broadcast([P, P]),
                        in1=iota_sb[:], op=mybir.AluOpType.is_equal)
                    nc.tensor.matmul(out=psA[:], lhsT=msg[:], rhs=sel[:],
                                     start=(k == 0), stop=(k == kt - 1))
                    e += 1
                aggT = aggp.tile([P, P], f32)
                nc.vector.tensor_copy(aggT[:], psA[:])
                psO = ps2.tile([P, dcol], f32, space="PSUM")
                nc.tensor.matmul(out=psO[:], lhsT=aggT[:],
                                 rhs=w_sb[layer][:, :dcol],
                                 start=True, stop=True)
                if layer < 3:
                    tmp = outp.tile([P, D], f32)
                    nc.scalar.activation(
                        out=tmp[:], in_=psO[:],
                        func=mybir.ActivationFunctionType.Relu,
                        scale=norm_sb[:, t:t + 1])
                    nc.vector.tensor_tensor(
                        out=h_next[:, t * P:(t + 1) * P], in0=tmp[:],
                        in1=norm_sb[:, t:t + 1].to_broadcast([P, P]),
                        op=mybir.AluOpType.mult)
                else:
                    ot = outp.tile([P, DOUT], f32)
                    nc.scalar.activation(
                        out=ot[:], in_=psO[:],
                        func=mybir.ActivationFunctionType.Copy,
                        scale=norm_sb[:, t:t + 1])
                    nc.gpsimd.dma_start(outd[t * P:(t + 1) * P, :], ot[:])
            if layer < 3:
                h_scaled = h_next
    nc.compile()
    return nc


def kernel(features, edge_index, W0, W1, W2, W3):
    features = np.asarray(features, dtype=np.float32)
    src = np.asarray(edge_index[0], dtype=np.int64)
    dst = np.asarray(edge_index[1], dtype=np.int64)
    Ws = [np.ascontiguousarray(np.asarray(w, dtype=np.float32))
          for w in (W0, W1, W2, W3)]

    deg = np.bincount(dst, minlength=N).astype(np.float32)
    norm = 1.0 / np.sqrt(np.maximum(deg, 1.0))

    # per-core edge partition by dst range; group edges by dst node-tile
    per_core = []
    cnt = np.zeros((NCORES, TPC), dtype=np.int64)
    for c in range(NCORES):
        m = (dst >= c * NPC) & (dst < (c + 1) * NPC)
        es = src[m]
        ed = dst[m] - c * NPC
        order = np.argsort(ed, kind="stable")
        es, ed = es[order], ed[order]
        tt = ed // P
        for t in range(TPC):
            cnt[c, t] = np.count_nonzero(tt == t)
        per_core.append((es, ed, tt))
    K = [max(1, int(-(-cnt[:, t].max() // P))) for t in range(TPC)]
    T = int(sum(K))

    in_maps = []
    iota = np.tile(np.arange(P, dtype=np.float32), (P, 1))
    for c in range(NCORES):
        es, ed, tt = per_core[c]
        src_col = np.zeros((T, P), dtype=np.int32)
        dst_col = np.full((T, P), -1.0, dtype=np.float32)
        col = 0
        for t in range(TPC):
            sel = tt == t
            s_t = es[sel]
            d_t = ed[sel] - t * P
            n = len(s_t)
            gidx = (s_t // NPC) * NPAD + (s_t % NPC)
            buf_s = np.zeros(K[t] * P, dtype=np.int32)
            buf_d = np.full(K[t] * P, -1.0, dtype=np.float32)
            buf_s[:n] = gidx
            buf_d[:n] = d_t.astype(np.float32)
            src_col[col:col + K[t]] = buf_s.reshape(K[t], P)
            dst_col[col:col + K[t]] = buf_d.reshape(K[t], P)
            col += K[t]
        nloc = np.zeros(NPAD, dtype=np.float32)
        nloc[:NPC] = norm[c * NPC:(c + 1) * NPC]
        feat_s = np.zeros((NPAD, D), dtype=np.float32)
        feat_s[:NPC] = features[c * NPC:(c + 1) * NPC] * nloc[:NPC, None]
        in_maps.append({
            "feat": feat_s.astype(_bf16np),
            "srcd": np.ascontiguousarray(src_col.T),
            "dstd": np.ascontiguousarray(dst_col.T),
            "normd": np.ascontiguousarray(nloc.reshape(TPC, P).T),
            "iotad": iota,
            "w0": Ws[0], "w1": Ws[1], "w2": Ws[2], "w3": Ws[3],
        })

    key = (tuple(K),)
    if key not in _cache:
        _cache[key] = _build(K, T)
    nc = _cache[key]
    global _last_in_maps
    _last_in_maps = in_maps
    res = run_bass_kernel_spmd(nc, in_maps, list(range(NCORES)))
    out = np.concatenate([res.results[c]["out"][:NPC] for c in range(NCORES)],
                         axis=0)
    return out.astype(np.float32)



# revision 4
# speedup vs baseline: 4.3947x; 4.3947x over previous
import sys, os
sys.path.insert(0, "/opt/trn_rl_repo")
import numpy as np
import ml_dtypes
_bf16np = ml_dtypes.bfloat16
_f8np = ml_dtypes.float8_e4m3

import jax
jax.config.update("jax_compilation_cache_dir", "/tmp/.bass_jax_cache")
jax.config.update("jax_persistent_cache_min_compile_time_secs", 0.0)
jax.config.update("jax_persistent_cache_min_entry_size_bytes", 0)

from contextlib import ExitStack
import concourse.tile as tile
from concourse import bass, bacc, mybir
from concourse.bass import IndirectOffsetOnAxis, ds
from concourse.bass_utils import run_bass_kernel_spmd

N = 50000
P = 128
NCORES = 8
NPC = N // NCORES          # 6250 nodes per core
TPC = (NPC + P - 1) // P   # 49 node tiles per core
NPAD = TPC * P             # 6272 padded nodes per core
NFULL = NCORES * NPAD      # 50176 padded gather-source rows
D = 128
DOUT = 40
WCOLS = 3 * D + DOUT       # packed weight columns

_cache = {}


def _build(KM):
    T = TPC * KM
    nc = bacc.Bacc("TRN2", target_bir_lowering=False, debug=False,
                   num_devices=NCORES)
    f32, i32, bf16 = mybir.dt.float32, mybir.dt.int32, mybir.dt.bfloat16
    u16, i8, f8 = mybir.dt.uint16, mybir.dt.int8, mybir.dt.float8e4

    feat = nc.dram_tensor("feat", [NPAD, D], f8, kind="ExternalInput").ap()
    srcd = nc.dram_tensor("srcd", [P, T], u16, kind="ExternalInput").ap()
    dstd = nc.dram_tensor("dstd", [P, T], i8, kind="ExternalInput").ap()
    normd = nc.dram_tensor("normd", [P, TPC], f32, kind="ExternalInput").ap()
    wd = nc.dram_tensor("w", [D, WCOLS], bf16, kind="ExternalInput").ap()
    outd = nc.dram_tensor("out", [NPAD, DOUT], bf16, kind="ExternalOutput").ap()

    with tile.TileContext(nc) as tc, ExitStack() as ctx:
        dram = ctx.enter_context(tc.tile_pool(name="dram", bufs=2, space="DRAM"))
        consts = ctx.enter_context(tc.tile_pool(name="consts", bufs=1))
        hpool = ctx.enter_context(tc.tile_pool(name="hs", bufs=2))
        msgp = ctx.enter_context(tc.tile_pool(name="msg", bufs=2 * KM))
        msgq = ctx.enter_context(tc.tile_pool(name="msgb", bufs=2 * KM))
        selp = ctx.enter_context(tc.tile_pool(name="sel", bufs=2 * KM))
        aggp = ctx.enter_context(tc.tile_pool(name="agg", bufs=4))
        outp = ctx.enter_context(tc.tile_pool(name="outp", bufs=4))
        ps1 = ctx.enter_context(tc.tile_pool(name="ps1", bufs=3, space="PSUM"))
        ps2 = ctx.enter_context(tc.tile_pool(name="ps2", bufs=3, space="PSUM"))

        src_u16 = consts.tile([P, T], u16)
        nc.gpsimd.dma_start(src_u16[:], srcd[:])
        src_sb = consts.tile([P, T], i32)
        nc.vector.tensor_copy(src_sb[:], src_u16[:])
        dst_i8 = consts.tile([P, T], i8)
        nc.gpsimd.dma_start(dst_i8[:], dstd[:])
        dst_sb = consts.tile([P, T], f32)
        nc.vector.tensor_copy(dst_sb[:], dst_i8[:])
        norm_sb = consts.tile([P, TPC], f32)
        nc.gpsimd.dma_start(norm_sb[:], normd[:])
        iota_i = consts.tile([P, P], i32)
        nc.gpsimd.iota(iota_i[:], [[1, P]], channel_multiplier=0)
        iota_sb = consts.tile([P, P], f32)
        nc.vector.tensor_copy(iota_sb[:], iota_i[:])
        w_sb = consts.tile([P, WCOLS], bf16)
        nc.gpsimd.dma_start(w_sb[:], wd[:])

        h_scaled = None
        for layer in range(4):
            dcol = D if layer < 3 else DOUT
            woff = layer * D
            hdt = f8 if layer == 0 else bf16
            hfull = dram.tile([NFULL, D], hdt, tag="hfull")
            if layer == 0:
                bounce0 = dram.tile([NPAD, D], f8, tag="bounce0")
                nc.gpsimd.dma_start(bounce0[:], feat[:])
                nc.gpsimd.collective_compute(
                    "AllGather", mybir.AluOpType.bypass,
                    replica_groups=[list(range(NCORES))],
                    ins=[bounce0.opt()], outs=[hfull.opt()],
                )
            else:
                bounce = dram.tile([NPAD, D], bf16, tag="bounce")
                # one transposing DMA: bounce[t*P+p, d] = h_scaled[p, t*P+d]
                bounce_t = bass.AP(bounce[:].tensor, 0,
                                   [[D, P], [P * D, TPC], [1, D]])
                nc.gpsimd.dma_start(bounce_t, h_scaled[:])
                nc.gpsimd.collective_compute(
                    "AllGather", mybir.AluOpType.bypass,
                    replica_groups=[list(range(NCORES))],
                    ins=[bounce.opt()], outs=[hfull.opt()],
                )
            if layer < 3:
                h_next = hpool.tile([P, NPAD], bf16)
            with tc.For_i(0, TPC, 1) as t:
                e0 = t * KM
                src_cur = aggp.tile([P, KM], i32, tag="srccur")
                nc.vector.tensor_copy(src_cur[:], src_sb[:, ds(e0, KM)])
                dst_cur = aggp.tile([P, KM], f32, tag="dstcur")
                nc.vector.tensor_copy(dst_cur[:], dst_sb[:, ds(e0, KM)])
                norm_cur = aggp.tile([P, 1], f32, tag="normcur")
                nc.vector.tensor_copy(norm_cur[:], norm_sb[:, ds(t, 1)])
                psA = ps1.tile([P, P], f32, space="PSUM")
                for k in range(KM):
                    msg = msgp.tile([P, D], hdt)
                    nc.gpsimd.indirect_dma_start(
                        out=msg[:], out_offset=None, in_=hfull[:],
                        in_offset=IndirectOffsetOnAxis(
                            ap=src_cur[:, k:k + 1], axis=0))
                    if layer == 0:
                        msgb = msgq.tile([P, D], bf16)
                        nc.vector.tensor_copy(msgb[:], msg[:])
                    else:
                        msgb = msg
                    sel = selp.tile([P, P], bf16)
                    nc.vector.tensor_tensor(
                        out=sel[:],
                        in0=dst_cur[:, k:k + 1].to_broadcast([P, P]),
                        in1=iota_sb[:], op=mybir.AluOpType.is_equal)
                    nc.tensor.matmul(out=psA[:], lhsT=msgb[:], rhs=sel[:],
                                     start=(k == 0), stop=(k == KM - 1))
                aggT = aggp.tile([P, P], bf16)
                nc.vector.tensor_copy(aggT[:], psA[:])
                psO = ps2.tile([P, dcol], f32, space="PSUM")
                nc.tensor.matmul(out=psO[:], lhsT=aggT[:],
                                 rhs=w_sb[:, woff:woff + dcol],
                                 start=True, stop=True)
                if layer < 3:
                    tmp = outp.tile([P, D], f32, tag="tmp")
                    nc.scalar.activation(
                        out=tmp[:], in_=psO[:],
                        func=mybir.ActivationFunctionType.Relu,
                        scale=norm_cur[:])
                    nc.vector.tensor_tensor(
                        out=h_next[:, ds(t * P, P)], in0=tmp[:],
                        in1=norm_cur[:].to_broadcast([P, P]),
                        op=mybir.AluOpType.mult)
                else:
                    ot = outp.tile([P, DOUT], bf16, tag="ot")
                    nc.scalar.activation(
                        out=ot[:], in_=psO[:],
                        func=mybir.ActivationFunctionType.Copy,
                        scale=norm_cur[:])
                    nc.gpsimd.dma_start(outd[ds(t * P, P), :], ot[:])
            if layer < 3:
                h_scaled = h_next
    nc.compile()
    return nc


def kernel(features, edge_index, W0, W1, W2, W3):
    features = np.asarray(features, dtype=np.float32)
    src = np.asarray(edge_index[0], dtype=np.int64)
    dst = np.asarray(edge_index[1], dtype=np.int64)
    wpack = np.concatenate(
        [np.asarray(w, dtype=np.float32) for w in (W0, W1, W2, W3)],
        axis=1).astype(_bf16np)

    deg = np.bincount(dst, minlength=N).astype(np.float32)
    norm = 1.0 / np.sqrt(np.maximum(deg, 1.0))

    # per-core edge partition by dst range; group edges by dst node-tile
    core = dst // NPC
    cnt = np.zeros((NCORES, TPC), dtype=np.int64)
    per_core = []
    for c in range(NCORES):
        m = core == c
        es = src[m]
        ed = dst[m] - c * NPC
        order = np.argsort(ed, kind="stable")
        es, ed = es[order], ed[order]
        tt = ed >> 7
        cnt[c] = np.bincount(tt, minlength=TPC)
        per_core.append((es, ed, tt))
    KM = max(1, int(-(-cnt.max() // P)))   # uniform batches per tile
    T = TPC * KM

    in_maps = []
    for c in range(NCORES):
        es, ed, tt = per_core[c]
        gidx = (es // NPC) * NPAD + (es % NPC)
        tile_start = np.zeros(TPC, dtype=np.int64)
        tile_start[1:] = np.cumsum(cnt[c])[:-1]
        pos = np.arange(len(es)) - tile_start[tt]
        slot = (tt * KM + (pos >> 7)) * P + (pos & 127)
        src_col = np.zeros(T * P, dtype=np.uint16)
        dst_col = np.full(T * P, -1, dtype=np.int8)
        src_col[slot] = gidx.astype(np.uint16)
        dst_col[slot] = (ed & 127).astype(np.int8)
        nloc = np.zeros(NPAD, dtype=np.float32)
        nloc[:NPC] = norm[c * NPC:(c + 1) * NPC]
        feat_s = np.zeros((NPAD, D), dtype=np.float32)
        feat_s[:NPC] = features[c * NPC:(c + 1) * NPC] * nloc[:NPC, None]
        in_maps.append({
            "feat": feat_s.astype(_f8np),
            "srcd": np.ascontiguousarray(src_col.reshape(T, P).T),
            "dstd": np.ascontiguousarray(dst_col.reshape(T, P).T),
            "normd": np.ascontiguousarray(nloc.reshape(TPC, P).T),
            "w": wpack,
        })

    key = (KM,)
    if key not in _cache:
        _cache[key] = _build(KM)
    nc = _cache[key]
    global _last_in_maps
    _last_in_maps = in_maps
    res = run_bass_kernel_spmd(nc, in_maps, list(range(NCORES)))
    out = np.concatenate([res.results[c]["out"][:NPC] for c in range(NCORES)],
                         axis=0)
    return out.astype(np.float32)
